# revision 4
# baseline (speedup 1.0000x reference)
"""Trainium2 Bass kernel for nn_ContextAwareModel.

Model: token embedding gather -> 2-layer "BiLSTM" (both directions scan
forward over time; 2 independent cells per layer) -> dense head.

Sharding (8 cores): core c handles cell ['f','b'][c//4] and batch shard
s = c%4 (rows 32s..32s+32).  The recurrence is communication-free under
(cell x batch-shard); layer-1 inputs x2 = [h1f, h1b] are exchanged with
per-block pair AllGathers ([[0,4],[1,5],[2,6],[3,7]]).

All large matmuls run in fp16 (10-bit mantissa, full PE rate; validated
~3.5e-4 rel error end to end), accumulation and elementwise in fp32.

Layout: everything transposed (gates.T, h.T, X.T) so batch rides the
matmul free dim and the 128-wide gate/hidden dims ride partitions.
"""

import numpy as np

B, T, V, EMB, H, SRC, NSRC = 128, 128, 50000, 1024, 512, 100, 3
D = EMB + 2 * H + SRC            # 2148
HALF = D // 2                    # 1074
N_CORES = 8
SH = 32                          # batch rows per core
NBLK = 8                         # time blocks
BT = T // NBLK                   # steps per block (16)
NM = 9                           # head m-tiles over 1074 (8*128 + 50)

_CACHE = {}


def _build():
    import concourse.bass as bass
    import concourse.mybir as mybir
    import concourse.tile as tile
    from concourse import bacc

    F16 = mybir.dt.float16
    F32 = mybir.dt.float32
    I32 = mybir.dt.int32
    AF = mybir.ActivationFunctionType

    nc = bacc.Bacc("TRN2", target_bir_lowering=False, debug=False,
                   num_devices=N_CORES)

    def din(name, shape, dt):
        return nc.dram_tensor(name, shape, dt, kind="ExternalInput").ap()

    E16 = din("E16", [V, EMB], F16)
    idx = din("idx", [128, 32], I32)
    wihT = [din(f"wih{l}T", [1024, 4 * H], F16) for l in range(2)]
    whhT = [din(f"whh{l}T", [H, 4 * H], F16) for l in range(2)]
    biasT = [din(f"bias{l}T", [128, 16], F32) for l in range(2)]
    tgtT = din("tgtT", [EMB, SH], F16)
    srcT = din("srcT", [SRC, SH], F16)
    dwtT = din("dwtT", [EMB, HALF], F16)
    dwfT = din("dwfT", [H, HALF], F16)
    dwbT = din("dwbT", [H, HALF], F16)
    dwsT = din("dwsT", [SRC, HALF], F16)
    dbT = din("dbT", [128, NM], F32)
    cwT = din("cwT", [HALF, 2], F16)
    cbT = din("cbT", [2, 1], F32)

    logitsT = nc.dram_tensor("logitsT", [2, SH], F32, kind="ExternalOutput").ap()
    probsT = nc.dram_tensor("probsT", [2, SH], F32, kind="ExternalOutput").ap()

    GROUPS = [[0, 4], [1, 5], [2, 6], [3, 7]]

    with tile.TileContext(nc) as tc:
        with tc.tile_pool(name="wpool", bufs=1) as wpool, \
             tc.tile_pool(name="xtp", bufs=2) as xtp, \
             tc.tile_pool(name="g0p", bufs=2) as g0p, \
             tc.tile_pool(name="hcp", bufs=2) as hcp, \
             tc.tile_pool(name="ewp", bufs=3) as ewp, \
             tc.tile_pool(name="xrowp", bufs=4) as xrowp, \
             tc.tile_pool(name="misc", bufs=1) as misc, \
             tc.tile_pool(name="headp", bufs=1) as headp, \
             tc.tile_pool(name="rec_ps", bufs=2, space="PSUM") as rec_ps, \
             tc.tile_pool(name="proj_ps", bufs=2, space="PSUM") as proj_ps, \
             tc.tile_pool(name="head_ps", bufs=2, space="PSUM") as head_ps, \
             tc.tile_pool(name="dram", bufs=1, space="DRAM") as dram:

            idx_sb = misc.tile([128, 32], I32)
            nc.sync.dma_start(idx_sb[:], idx[:])

            # DRAM intermediates
            Xg = [dram.tile([4 * 128, EMB], F16, tag=f"xg{b % 2}", name=f"Xg{b}") for b in range(NBLK)]
            H1self = [dram.tile([H, 512], F16, tag=f"h1s{b}", name=f"H1self{b}") for b in range(NBLK)]
            H1full = [dram.tile([2 * H, 512], F16, tag=f"h1f{b}", name=f"H1full{b}") for b in range(NBLK)]
            h2selfD = dram.tile([H, SH], F16, tag="h2s")
            h2fullD = dram.tile([2 * H, SH], F16, tag="h2f")

            def emit_gathers(blk):
                for g in range(4):
                    xrow = xrowp.tile([128, EMB], F16, tag="xrow")
                    nc.gpsimd.indirect_dma_start(
                        out=xrow[:], out_offset=None, in_=E16[:],
                        in_offset=bass.IndirectOffsetOnAxis(
                            ap=idx_sb[:, 4 * blk + g: 4 * blk + g + 1], axis=0),
                    )
                    nc.sync.dma_start(Xg[blk][bass.ts(g, 128), :], xrow[:])

            def emit_rhs(lay, blk):
                """Produce xt tile [128, 8, 512] fp16 (X.T or H1.T chunk)."""
                xt = xtp.tile([128, 8, 512], F16, tag="xt")
                if lay == 0:
                    for k in range(8):
                        nc.sync.dma_start_transpose(
                            out=xt[:, k], in_=Xg[blk][:, bass.ts(k, 128)])
                else:
                    nc.sync.dma_start(
                        xt[:],
                        H1full[blk].rearrange("(k p) c -> p k c", p=128))
                return xt

            def emit_proj_mtile(wih_sb, bias_sb, xt, g0, m):
                ps = proj_ps.tile([128, 512], F32, tag="proj")
                for k in range(8):
                    nc.tensor.matmul(ps[:], wih_sb[:, k, bass.ts(m, 128)],
                                     xt[:, k], start=(k == 0), stop=(k == 7))
                nc.scalar.activation(
                    g0[:, :, bass.ts(m, 32)],
                    ps[:].rearrange("p (t b) -> p t b", b=32),
                    AF.Identity, bias=bias_sb[:, m:m + 1])

            def load_layer_weights(lay):
                wih_sb = wpool.tile([128, 8, 4 * H], F16, tag="wih")
                nc.sync.dma_start(
                    wih_sb[:], wihT[lay].rearrange("(k p) g -> p k g", p=128))
                whh_sb = wpool.tile([128, 4, 4 * H], F16, tag="whh")
                nc.sync.dma_start(
                    whh_sb[:], whhT[lay].rearrange("(k p) g -> p k g", p=128))
                bias_sb = wpool.tile([128, 16], F32, tag="bias")
                nc.sync.dma_start(bias_sb[:], biasT[lay][:])
                return wih_sb, whh_sb, bias_sb

            def emit_step(lay, blk, tl, whh_sb, g0, h_prev, c_prev):
                ps = rec_ps.tile([128, 512], F32, tag="rec")
                for m in range(16):
                    for k in range(4):
                        nc.tensor.matmul(
                            ps[:, bass.ts(m, 32)],
                            whh_sb[:, k, bass.ts(m, 128)],
                            h_prev[:, bass.ts(k, 32)],
                            start=(k == 0), stop=(k == 3))
                # gates = ps + g0[t]; regions i,f,g,o = cols 0:128,...
                gsl = g0[:, tl]
                gi = ewp.tile([128, 128], F32, tag="gi")
                nc.vector.tensor_add(gi[:], ps[:, 0:128], gsl[:, 0:128])
                gf = ewp.tile([128, 128], F32, tag="gf")
                nc.vector.tensor_add(gf[:], ps[:, 128:256], gsl[:, 128:256])
                gg = ewp.tile([128, 128], F32, tag="gg")
                nc.vector.tensor_add(gg[:], ps[:, 256:384], gsl[:, 256:384])
                go = ewp.tile([128, 128], F32, tag="go")
                nc.vector.tensor_add(go[:], ps[:, 384:512], gsl[:, 384:512])
                si = ewp.tile([128, 128], F32, tag="si")
                nc.scalar.activation(si[:], gi[:], AF.Sigmoid)
                sf = ewp.tile([128, 128], F32, tag="sf")
                nc.scalar.activation(sf[:], gf[:], AF.Sigmoid)
                tg = ewp.tile([128, 128], F32, tag="tg")
                nc.scalar.activation(tg[:], gg[:], AF.Tanh)
                so = ewp.tile([128, 128], F32, tag="so")
                nc.scalar.activation(so[:], go[:], AF.Sigmoid)
                fc = ewp.tile([128, 128], F32, tag="fc")
                nc.vector.tensor_mul(fc[:], sf[:], c_prev[:])
                it = ewp.tile([128, 128], F32, tag="it")
                nc.vector.tensor_mul(it[:], si[:], tg[:])
                c_new = hcp.tile([128, 128], F32, tag="c")
                nc.vector.tensor_add(c_new[:], fc[:], it[:])
                tc_ = ewp.tile([128, 128], F32, tag="tc")
                nc.scalar.activation(tc_[:], c_new[:], AF.Tanh)
                h_new = hcp.tile([128, 128], F16, tag="h")
                nc.vector.tensor_mul(h_new[:], so[:], tc_[:])
                t_glob = BT * blk + tl
                if lay == 0:
                    nc.sync.dma_start(
                        H1self[blk].rearrange("(j p) c -> p j c", p=128)
                        [:, :, bass.ts(tl, 32)],
                        h_new[:].rearrange("p (j b) -> p j b", j=4))
                elif t_glob == T - 1:
                    nc.sync.dma_start(
                        h2selfD.rearrange("(j p) c -> p j c", p=128),
                        h_new[:].rearrange("p (j b) -> p j b", j=4))
                return h_new, c_new

            # ================= layer passes =================
            for lay in range(2):
                wih_sb, whh_sb, bias_sb = load_layer_weights(lay)
                if lay == 0:
                    emit_gathers(0)
                xt_cur = emit_rhs(lay, 0)
                g0_cur = g0p.tile([128, BT, 512], F32, tag="g0")
                for m in range(16):
                    emit_proj_mtile(wih_sb, bias_sb, xt_cur, g0_cur, m)

                h = hcp.tile([128, 128], F16, tag="h")
                nc.vector.memset(h[:], 0.0)
                c = hcp.tile([128, 128], F32, tag="c")
                nc.vector.memset(c[:], 0.0)

                for blk in range(NBLK):
                    have_next = blk + 1 < NBLK
                    if have_next:
                        if lay == 0:
                            emit_gathers(blk + 1)
                        xt_next = emit_rhs(lay, blk + 1)
                        g0_next = g0p.tile([128, BT, 512], F32, tag="g0")
                    for tl in range(BT):
                        h, c = emit_step(lay, blk, tl, whh_sb, g0_cur, h, c)
                        if have_next:
                            emit_proj_mtile(wih_sb, bias_sb, xt_next,
                                            g0_next, tl)
                    if lay == 0:
                        nc.gpsimd.collective_compute(
                            "AllGather", mybir.AluOpType.bypass,
                            replica_groups=GROUPS,
                            ins=[H1self[blk].opt()], outs=[H1full[blk].opt()])
                    if have_next:
                        xt_cur, g0_cur = xt_next, g0_next

            # final h2 exchange
            nc.gpsimd.collective_compute(
                "AllGather", mybir.AluOpType.bypass, replica_groups=GROUPS,
                ins=[h2selfD.opt()], outs=[h2fullD.opt()])

            # ================= head (replicated on every core) ===========
            dwt_sb = headp.tile([128, 8, HALF], F16, tag="dwt")
            nc.sync.dma_start(dwt_sb[:], dwtT.rearrange("(k p) m -> p k m", p=128))
            dwf_sb = headp.tile([128, 4, HALF], F16, tag="dwf")
            nc.sync.dma_start(dwf_sb[:], dwfT.rearrange("(k p) m -> p k m", p=128))
            dwb_sb = headp.tile([128, 4, HALF], F16, tag="dwb")
            nc.sync.dma_start(dwb_sb[:], dwbT.rearrange("(k p) m -> p k m", p=128))
            dws_sb = headp.tile([128, HALF], F16, tag="dws")
            nc.sync.dma_start(dws_sb[:SRC, :], dwsT[:])
            tgt_sb = headp.tile([128, 8, SH], F16, tag="tgt")
            nc.sync.dma_start(tgt_sb[:], tgtT.rearrange("(k p) b -> p k b", p=128))
            src_sb = headp.tile([128, SH], F16, tag="src")
            nc.sync.dma_start(src_sb[:SRC, :], srcT[:])
            h2_sb = headp.tile([128, 8, SH], F16, tag="h2")
            nc.sync.dma_start(h2_sb[:], h2fullD.rearrange("(k p) b -> p k b", p=128))
            db_sb = headp.tile([128, NM], F32, tag="db")
            nc.sync.dma_start(db_sb[:], dbT[:])
            cw_sb = headp.tile([128, NM, 2], F16, tag="cw")
            for m in range(NM):
                mm = min(128, HALF - 128 * m)
                nc.sync.dma_start(cw_sb[:mm, m, :], cwT[128 * m:128 * m + mm, :])
            cb_sb = headp.tile([2, 1], F32, tag="cb")
            nc.sync.dma_start(cb_sb[:], cbT[:])

            featT = headp.tile([128, NM, SH], F16, tag="feat")
            for m in range(NM):
                mm = min(128, HALF - 128 * m)
                fps = head_ps.tile([128, 512], F32, tag="fps")
                chunks = ([(dwt_sb[:, k, :], tgt_sb[:, k, :]) for k in range(8)]
                          + [(dwf_sb[:, k, :], h2_sb[:, k, :]) for k in range(4)]
                          + [(dwb_sb[:, k, :], h2_sb[:, 4 + k, :]) for k in range(4)]
                          + [(dws_sb[:SRC, :], src_sb[:SRC, :])])
                nck = len(chunks)
                for ki, (lh, rh) in enumerate(chunks):
                    nc.tensor.matmul(fps[:mm, 0:SH],
                                     lh[:, 128 * m:128 * m + mm], rh,
                                     start=(ki == 0), stop=(ki == nck - 1))
                nc.scalar.activation(featT[:mm, m, :], fps[:mm, 0:SH],
                                     AF.Tanh, bias=db_sb[:mm, m:m + 1])

            lps = head_ps.tile([2, SH], F32, tag="lps")
            for m in range(NM):
                mm = min(128, HALF - 128 * m)
                nc.tensor.matmul(lps[:], cw_sb[:mm, m, :], featT[:mm, m, :],
                                 start=(m == 0), stop=(m == NM - 1))
            lg_sb = headp.tile([2, SH], F32, tag="lg")
            nc.scalar.activation(lg_sb[:], lps[:], AF.Identity, bias=cb_sb[:])
            pb_sb = headp.tile([2, SH], F32, tag="pb")
            nc.scalar.activation(pb_sb[:], lps[:], AF.Sigmoid, bias=cb_sb[:])
            nc.sync.dma_start(logitsT[:], lg_sb[:])
            nc.sync.dma_start(probsT[:], pb_sb[:])

    nc.compile()
    return nc


def _prep_inputs(inputs):
    """Host-side sharding/layout prep. Returns (in_maps, tgt_rows_f32)."""
    article = np.asarray(inputs["article"]).astype(np.int64)
    positions = np.asarray(inputs["positions"]).astype(np.int64)
    srcs = np.asarray(inputs["srcs"]).astype(np.int64)
    E = np.asarray(inputs["E"], dtype=np.float32)
    E16 = E.astype(np.float16)
    src_emb = np.asarray(inputs["src_emb"], dtype=np.float32)

    tgt_rows = E[article[np.arange(B), positions]]          # [B, EMB] fp32
    src_rows = src_emb[srcs]                                # [B, SRC]

    f16 = np.float16
    w = {k: np.asarray(v, dtype=np.float32) for k, v in inputs.items()
         if k not in ("article", "positions", "srcs", "E")}

    dW = w["dW"]
    common = dict(
        E16=E16,
        dwtT=np.ascontiguousarray(dW[:, :EMB].T.astype(f16)),
        dwfT=np.ascontiguousarray(dW[:, EMB:EMB + H].T.astype(f16)),
        dwbT=np.ascontiguousarray(dW[:, EMB + H:EMB + 2 * H].T.astype(f16)),
        dwsT=np.ascontiguousarray(dW[:, EMB + 2 * H:].T.astype(f16)),
        dbT=_pad_cols(w["db"], NM * 128).reshape(NM, 128).T.copy(),
        cwT=np.ascontiguousarray(w["cW"].T.astype(f16)),
        cbT=w["cb"].reshape(2, 1).astype(np.float32),
    )

    in_maps = []
    for c in range(N_CORES):
        cell = "f" if c < 4 else "b"
        s = c % 4
        rows = slice(SH * s, SH * (s + 1))
        art_s = article[rows]                               # [32, T]
        r = np.arange(128 * 32)                             # gathers flattened
        # idx[p, g]: row (128g+p) of X = (t, b) with t=(128g+p)//32, b=%32
        gp = (np.arange(32)[None, :] * 128 + np.arange(128)[:, None])
        idx_arr = art_s[gp % 32, gp // 32].astype(np.int32)

        m = dict(common)
        m["idx"] = idx_arr
        for lay in range(2):
            wih = w[f"Wih{lay}{cell}"]
            whh = w[f"Whh{lay}{cell}"]
            bias = w[f"bih{lay}{cell}"] + w[f"bhh{lay}{cell}"]
            m[f"wih{lay}T"] = np.ascontiguousarray(wih.T.astype(f16))
            m[f"whh{lay}T"] = np.ascontiguousarray(whh.T.astype(f16))
            m[f"bias{lay}T"] = bias.reshape(16, 128).T.copy()
        m["tgtT"] = np.ascontiguousarray(tgt_rows[rows].T.astype(f16))
        m["srcT"] = np.ascontiguousarray(src_rows[rows].T.astype(f16))
        in_maps.append(m)
    return in_maps, tgt_rows


def _pad_cols(v, n):
    out = np.zeros(n, dtype=np.float32)
    out[:v.shape[0]] = v
    return out


def _run(inputs, trace=False, tmpdir=None):
    from concourse import bass_utils
    if "nc" not in _CACHE:
        _CACHE["nc"] = _build()
    nc = _CACHE["nc"]
    in_maps, tgt_rows = _prep_inputs(inputs)
    res = bass_utils.run_bass_kernel_spmd(
        nc, in_maps, core_ids=list(range(N_CORES)), trace=trace,
        **({"tmpdir": tmpdir} if tmpdir else {}))
    logits = np.concatenate(
        [res.results[s]["logitsT"].T for s in range(4)], axis=0)
    probs = np.concatenate(
        [res.results[s]["probsT"].T for s in range(4)], axis=0)
    return (logits.astype(np.float32), probs.astype(np.float32),
            tgt_rows.astype(np.float32)), res


def kernel(**inputs):
    out, _ = _run(inputs, trace=False)
    return out
